# revision 24
# baseline (speedup 1.0000x reference)
"""Trainium2 Bass kernel for nn_MultiHeadedAttention_4269197492266.

Dual-branch multi-head attention where the "local" key path is a multi-scale
conv (k=3,5) + batchnorm + projection.  Host-side algebra folds the whole
local path into a single 5-tap convolution:

    kl = bn(concat(conv3(key), conv5(key))) @ wkl.T + bkl
       = conv5tap(key, W5c) + bkl_eff

with W5c[o,i,d] = A5[o,i,d] + A3[o,i,d-1] (A* = wkl-slice @ (bn_scale * conv_w*)).
This makes every tensor needed by head h a single-stage (shifted) matmul of the
raw inputs, so work shards cleanly over (batch, head-group) with no
collectives: core c handles batch c//2, heads 4*(c%2) .. 4*(c%2)+4.  Each core
emits the partial output projection of its 4 heads; the host adds the two
partials per batch plus the folded bias.

On-chip layout is feature-major ([d, L]).  Scores are computed transposed
([Lk, Lq]) so the AV matmul needs no transposes; a ones-column appended to V
makes the softmax denominator fall out of the same PSUM accumulation
(row 64), normalized via reciprocal + gpsimd partition_broadcast.
"""

import math
from contextlib import ExitStack

import ml_dtypes
import numpy as np

import concourse.tile as tile
from concourse import bacc, mybir
from concourse import bass_utils

F32 = mybir.dt.float32
BF16 = mybir.dt.bfloat16
BF16_NP = ml_dtypes.bfloat16

B, L, D = 4, 2048, 512
H, DK = 8, 64
N_CORES = 8
HG = 4              # heads per core
DO = HG * DK        # 256 output dims per core
BN_EPS = 1e-5
NJ = D // 128       # 4 input-dim tiles
NLT = L // 128      # 16 L tiles of 128
NLQ = L // 512      # 4 lq blocks of 512
NCH = L // 1024     # 2 score chunks of 1024 per lk tile

ET_BUFS = 28

_cache = {}


def _build_program(repeat=1, stages='all'):
    """Build + compile the per-core Bass program (same program on all cores)."""
    nc = bacc.Bacc("TRN2", target_bir_lowering=False, debug=False,
                   num_devices=N_CORES)

    dt_in = {}
    dt_in["xq"] = nc.dram_tensor("xq", [D, L], BF16, kind="ExternalInput").ap()
    dt_in["xk"] = nc.dram_tensor("xk", [D, L], BF16, kind="ExternalInput").ap()
    dt_in["xv"] = nc.dram_tensor("xv", [D, L], BF16, kind="ExternalInput").ap()
    dt_in["wq"] = nc.dram_tensor("wq", [D, DO], BF16, kind="ExternalInput").ap()
    dt_in["wk5"] = nc.dram_tensor("wk5", [5, D, DO], BF16, kind="ExternalInput").ap()
    dt_in["wkg"] = nc.dram_tensor("wkg", [D, DO], BF16, kind="ExternalInput").ap()
    dt_in["wv"] = nc.dram_tensor("wv", [D, DO], BF16, kind="ExternalInput").ap()
    dt_in["wo"] = nc.dram_tensor("wo", [DO, D], BF16, kind="ExternalInput").ap()
    dt_in["bkl"] = nc.dram_tensor("bkl", [DO], F32, kind="ExternalInput").ap()
    out_ap = nc.dram_tensor("out", [L, D], F32, kind="ExternalOutput").ap()

    with tile.TileContext(nc) as tc, ExitStack() as ctx:
        big = ctx.enter_context(tc.tile_pool(name="big", bufs=12))
        et = ctx.enter_context(tc.tile_pool(name="et", bufs=ET_BUFS))
        proj = ctx.enter_context(tc.tile_pool(name="projsb", bufs=1))
        norm = ctx.enter_context(tc.tile_pool(name="norm", bufs=2))
        ostage = ctx.enter_context(tc.tile_pool(name="ostage", bufs=2))
        sp = ctx.enter_context(tc.tile_pool(name="sp", bufs=2, space="PSUM"))
        work = ctx.enter_context(tc.tile_pool(name="work", bufs=4, space="PSUM"))

        # ---- persistent SBUF tensors (single-buffer pools) ----
        wq_sb = proj.tile([128, NJ, DO], BF16, tag="wq")
        wk5_sb = proj.tile([128, 5, NJ, DO], BF16, tag="wk5")
        wkg_sb = proj.tile([128, NJ, DO], BF16, tag="wkg")
        wv_sb = proj.tile([128, NJ, DO], BF16, tag="wv")
        wo_sb = proj.tile([64, HG, D], BF16, tag="wo")
        bkl_sb = proj.tile([128, 2], F32, tag="bkl")
        qT_sb = proj.tile([128, 2, L], BF16, tag="qT")
        klT_sb = proj.tile([128, 2, L], BF16, tag="klT")
        kgT_sb = proj.tile([128, 2, L], BF16, tag="kgT")
        v_sb = proj.tile([128, NLT, HG, DK + 1], BF16, tag="v")
        xT_sb = [proj.tile([64, HG, L], BF16, tag=f"xT{br}", name=f"xT{br}")
                 for br in range(2)]

        nc.sync.dma_start(wq_sb[:], dt_in["wq"].rearrange("(j p) o -> p j o", p=128))
        nc.sync.dma_start(wkg_sb[:], dt_in["wkg"].rearrange("(j p) o -> p j o", p=128))

        warm = proj.tile([1, 16], F32, tag="warm")
        nc.vector.memset(warm[:], 0.0)
        nc.scalar.activation(warm[:], warm[:], mybir.ActivationFunctionType.Exp)

        def emit_body():
            # ---- load activations (feature-major), key padded for the conv ----
            LKP = L + 4  # padded length
            kx = []
            for j in range(NJ):
                t = big.tile([128, LKP], BF16, tag="big")
                nc.vector.memset(t[:, 0:2], 0.0)
                nc.vector.memset(t[:, 2 + L:], 0.0)
                nc.sync.dma_start(t[:, 2:2 + L], dt_in["xk"][j * 128:(j + 1) * 128, :])
                kx.append(t)
            xq = []
            xv = []
            for j in range(NJ):
                t = big.tile([128, LKP], BF16, tag="big")
                nc.gpsimd.dma_start(t[:, :L], dt_in["xq"][j * 128:(j + 1) * 128, :])
                xq.append(t)
            nc.sync.dma_start(wk5_sb[:], dt_in["wk5"].rearrange("t (j p) o -> p t j o", p=128))
            nc.sync.dma_start(bkl_sb[:], dt_in["bkl"].rearrange("(m p) -> p m", p=128))
            nc.sync.dma_start(wv_sb[:], dt_in["wv"].rearrange("(j p) o -> p j o", p=128))
            nc.sync.dma_start(wo_sb[:], dt_in["wo"].rearrange("(h p) o -> p h o", p=64))
            for j in range(NJ):
                t = big.tile([128, LKP], BF16, tag="big")
                nc.sync.dma_start(t[:, :L], dt_in["xv"][j * 128:(j + 1) * 128, :])
                xv.append(t)

            # ---- projections ----
            def proj_chunk(dst_sb, w_sb, m, qb, src, bias=None, off=0):
                # dst [do=128 partitions, 512 of L]:  accumulate over NJ input tiles
                ps = work.tile([128, 512], F32, tag="wk")
                for j in range(NJ):
                    nc.tensor.matmul(ps[:], w_sb[:, j, m * 128:(m + 1) * 128],
                                     src[j][:, off + qb * 512:off + qb * 512 + 512],
                                     start=(j == 0), stop=(j == NJ - 1))
                if bias is not None:
                    nc.vector.tensor_scalar_add(
                        dst_sb[:, m, qb * 512:qb * 512 + 512], ps[:], bias[:, m:m + 1])
                else:
                    nc.vector.tensor_copy(dst_sb[:, m, qb * 512:qb * 512 + 512], ps[:])

            def klT_chunk(m, qb):
                # 5-tap conv projection: shifted slices of padded key
                ps = work.tile([128, 512], F32, tag="wk")
                first = True
                for t in range(5):
                    for j in range(NJ):
                        sh = qb * 512 + t  # (t-2) shift + 2 pad offset
                        nc.tensor.matmul(ps[:], wk5_sb[:, t, j, m * 128:(m + 1) * 128],
                                         kx[j][:, sh:sh + 512],
                                         start=first, stop=(t == 4 and j == NJ - 1))
                        first = False
                nc.vector.tensor_scalar_add(
                    klT_sb[:, m, qb * 512:qb * 512 + 512], ps[:], bkl_sb[:, m:m + 1])

            def v_proj(lts=None):
                if lts is None or lts.start == 0:
                    nc.vector.memset(v_sb[:], 1.0)
                for lt in (range(NLT) if lts is None else lts):
                    ps = work.tile([128, 512], F32, tag="wk")
                    for j in range(NJ):
                        nc.tensor.matmul(ps[:, :DO], xv[j][:, lt * 128:lt * 128 + 128],
                                         wv_sb[:, j, :],
                                         start=(j == 0), stop=(j == NJ - 1))
                    nc.vector.tensor_copy(
                        v_sb[:, lt, :, 0:DK],
                        ps[:, :DO].rearrange("p (h d) -> p h d", h=HG))

            def scores_exp_pair(p, br, qh, lks=None, eT=None, cs=None):
                # two heads (2p at partitions 0-63, 2p+1 at 64-127) computed
                # by row-disjoint concurrent matmuls into one psum tile;
                # covers lq half qh (chunks 2qh, 2qh+1)
                kT = klT_sb if br == 0 else kgT_sb
                if eT is None:
                    eT = {}
                for lk in (range(NLT) if lks is None else lks):
                    for c in (cs if cs is not None else (2 * qh, 2 * qh + 1)):
                        ps = sp.tile([128, 1024], F32, tag="sp")
                        for hh in range(2):
                            pb = 64 * hh
                            nc.tensor.matmul(
                                ps[:, hh * 512:hh * 512 + 512],
                                kT[pb:pb + 64, p, lk * 128:lk * 128 + 128],
                                qT_sb[pb:pb + 64, p, c * 512:c * 512 + 512],
                                start=True, stop=True)
                        e_t = et.tile([128, 1024], BF16, tag="et")
                        nc.scalar.activation(e_t[:], ps[:],
                                             mybir.ActivationFunctionType.Exp)
                        eT[(lk, c)] = e_t
                return eT

            def av_norm_pair(p, br, qh, eT):
                # both heads accumulate concurrently (4 psum accumulators),
                # so eT tiles release progressively along lk
                avs = [[work.tile([DK + 1, 512], F32, tag="wk",
                                  name=f"av{hh}_{i}") for i in range(2)]
                       for hh in range(2)]
                for lk in range(NLT):
                    for hh in range(2):
                        h = 2 * p + hh
                        for i in range(2):
                            c = 2 * qh + i
                            nc.tensor.matmul(
                                avs[hh][i][:], v_sb[:, lk, h, :],
                                eT[(lk, c)][:, hh * 512:hh * 512 + 512],
                                start=(lk == 0), stop=(lk == NLT - 1))
                for hh in range(2):
                    h = 2 * p + hh
                    for i in range(2):
                        c = 2 * qh + i
                        av = avs[hh][i]
                        rd = norm.tile([DK + 1, 512], F32, tag="rd")
                        nc.vector.reciprocal(rd[DK:DK + 1, :], av[DK:DK + 1, :])
                        # HW partition_broadcast reads absolute partition 0:
                        # DMA-remap the row 64 -> 0 first.
                        r0 = norm.tile([1, 512], F32, tag="r0")
                        nc.sync.dma_start(r0[:], rd[DK:DK + 1, :])
                        bc = norm.tile([DK, 512], F32, tag="bc")
                        nc.gpsimd.partition_broadcast(bc[:], r0[0:1, :])
                        nc.vector.tensor_tensor(
                            xT_sb[br][:, h, c * 512:c * 512 + 512],
                            av[0:DK, :], bc[:], mybir.AluOpType.mult)

            def av_alloc_c(tag):
                return [work.tile([DK + 1, 512], F32, tag="wk",
                                  name=f"avc{tag}{hh}") for hh in range(2)]

            def av_mms_c(avs2, p, c, eT, lks):
                for lk in lks:
                    for hh in range(2):
                        nc.tensor.matmul(
                            avs2[hh][:], v_sb[:, lk, 2 * p + hh, :],
                            eT[(lk, c)][:, hh * 512:hh * 512 + 512],
                            start=(lk == 0), stop=(lk == NLT - 1))

            def av_norms_c(avs2, p, br, c):
                for hh in range(2):
                    h = 2 * p + hh
                    av = avs2[hh]
                    rd = norm.tile([DK + 1, 512], F32, tag="rd")
                    nc.vector.reciprocal(rd[DK:DK + 1, :], av[DK:DK + 1, :])
                    r0 = norm.tile([1, 512], F32, tag="r0")
                    nc.sync.dma_start(r0[:], rd[DK:DK + 1, :])
                    bc = norm.tile([DK, 512], F32, tag="bc")
                    nc.gpsimd.partition_broadcast(bc[:], r0[0:1, :])
                    nc.vector.tensor_tensor(
                        xT_sb[br][:, h, c * 512:c * 512 + 512],
                        av[0:DK, :], bc[:], mybir.AluOpType.mult)

            def outproj_tile(lt):
                po = work.tile([128, 512], F32, tag="wk")
                k = 0
                for br in range(2):
                    for h in range(HG):
                        nc.tensor.matmul(
                            po[:], xT_sb[br][:, h, lt * 128:lt * 128 + 128],
                            wo_sb[:, h, :],
                            start=(k == 0), stop=(k == 2 * HG - 1))
                        k += 1
                ot = ostage.tile([128, D], F32, tag="ot")
                nc.vector.tensor_copy(ot[:], po[:])
                nc.sync.dma_start(out_ap[lt * 128:lt * 128 + 128, :], ot[:])

            def outproj_half(qh):
                for lt in range(8 * qh, 8 * qh + 8):
                    outproj_tile(lt)

            # ---- interleaved emission ----
            proj_chunk(kgT_sb, wkg_sb, 0, 0, kx, off=2)
            for qb in range(2):
                proj_chunk(qT_sb, wq_sb, 0, qb, xq)
            if stages == 'proj':
                for qb in range(2, NLQ):
                    proj_chunk(qT_sb, wq_sb, 0, qb, xq)
                for qb in range(NLQ):
                    klT_chunk(0, qb)
                for qb in range(1, NLQ):
                    proj_chunk(kgT_sb, wkg_sb, 0, qb, kx, off=2)
                v_proj()
                for qb in range(NLQ):
                    proj_chunk(qT_sb, wq_sb, 1, qb, xq)
                    klT_chunk(1, qb)
                    proj_chunk(kgT_sb, wkg_sb, 1, qb, kx, off=2)
                return
            do_av = stages != 'scores'

            # software pipeline: scores/exp of phase k+1 emitted before the
            # AV of phase k, so PE always has AV work while ACT streams exp.
            # Global branch first: its key projection (kgT) is 5x cheaper
            # than the folded-conv klT, so the exp stream starts earliest.
            e1 = {}
            scores_exp_pair(0, 1, 0, lks=range(0, 4), eT=e1)
            for qb in range(1, NLQ):
                proj_chunk(kgT_sb, wkg_sb, 0, qb, kx, off=2)
                scores_exp_pair(0, 1, 0, lks=range(4 * qb, 4 * qb + 4), eT=e1)
            e2 = {}
            for qb in range(NLQ):
                klT_chunk(0, qb)
                scores_exp_pair(0, 0, 0, lks=range(4 * qb, 4 * qb + 4), eT=e2)
            v_proj()
            if do_av:
                av_norm_pair(0, 1, 0, e1)
            for qb in range(2, NLQ):
                proj_chunk(qT_sb, wq_sb, 0, qb, xq)
            for qb in range(2):
                proj_chunk(qT_sb, wq_sb, 1, qb, xq)
            e3 = {}
            for qb in range(NLQ):
                proj_chunk(kgT_sb, wkg_sb, 1, qb, kx, off=2)
                scores_exp_pair(1, 1, 0, lks=range(4 * qb, 4 * qb + 4), eT=e3)
                if qb < 2:
                    klT_chunk(1, qb)
            if do_av:
                av_norm_pair(0, 0, 0, e2)
            e4 = {}
            for qb in range(NLQ):
                if qb >= 2:
                    klT_chunk(1, qb)
                scores_exp_pair(1, 0, 0, lks=range(4 * qb, 4 * qb + 4), eT=e4)
            if do_av:
                av_norm_pair(1, 1, 0, e3)
            for qb in range(2, NLQ):
                proj_chunk(qT_sb, wq_sb, 1, qb, xq)
            e5 = scores_exp_pair(0, 1, 1)
            if do_av:
                av_norm_pair(1, 0, 0, e4)
            e6 = scores_exp_pair(0, 0, 1)
            if do_av:
                av_norm_pair(0, 1, 1, e5)
                outproj_tile(0)
                outproj_tile(1)
            e7 = scores_exp_pair(1, 1, 1)
            if do_av:
                av_norm_pair(0, 0, 1, e6)
                outproj_tile(2)
                outproj_tile(3)
            # final phase split into two lq-quarter mini-phases with 2-slot
            # AV accumulators, so the last AV trails its own exp stream
            # instead of running as a serial tail burst
            e8 = {}
            if do_av:
                av7 = [[None, None], [None, None]]
                av7f = [work.tile([DK + 1, 512], F32, tag="wk", name=f"av7_{z}")
                        for z in range(4)]
                for qb in range(NLQ):
                    scores_exp_pair(1, 0, 1, lks=range(4 * qb, 4 * qb + 4),
                                    eT=e8, cs=(2,))
                    for lk in range(4 * qb, 4 * qb + 4):
                        for hh in range(2):
                            for i in range(2):
                                nc.tensor.matmul(
                                    av7f[2 * hh + i][:],
                                    v_sb[:, lk, 2 * 1 + hh, :],
                                    e7[(lk, 2 + i)][:, hh * 512:hh * 512 + 512],
                                    start=(lk == 0), stop=(lk == NLT - 1))
                for hh in range(2):
                    for i in range(2):
                        avs2v = av7f[2 * hh + i]
                        rd = norm.tile([DK + 1, 512], F32, tag="rd")
                        nc.vector.reciprocal(rd[DK:DK + 1, :], avs2v[DK:DK + 1, :])
                        r0 = norm.tile([1, 512], F32, tag="r0")
                        nc.sync.dma_start(r0[:], rd[DK:DK + 1, :])
                        bc = norm.tile([DK, 512], F32, tag="bc")
                        nc.gpsimd.partition_broadcast(bc[:], r0[0:1, :])
                        nc.vector.tensor_tensor(
                            xT_sb[1][:, 2 + hh, (2 + i) * 512:(2 + i) * 512 + 512],
                            avs2v[0:DK, :], bc[:], mybir.AluOpType.mult)
                for lt in (4, 5, 6, 7):
                    outproj_tile(lt)
                av8a = av_alloc_c("a")
                av8b = av_alloc_c("b")
                for qb in range(NLQ):
                    scores_exp_pair(1, 0, 1, lks=range(4 * qb, 4 * qb + 4),
                                    eT=e8, cs=(3,))
                    av_mms_c(av8a, 1, 2, e8, range(4 * qb, 4 * qb + 4))
                    av_mms_c(av8b, 1, 3, e8, range(4 * qb, 4 * qb + 4))
                av_norms_c(av8a, 1, 0, 2)
                av_norms_c(av8b, 1, 0, 3)
                outproj_half(1)
            else:
                scores_exp_pair(1, 0, 1, eT=e8)

        for _rep in range(repeat):
            emit_body()

    nc.compile()
    return nc


def _host_prep(inputs):
    """Fold conv+bn+biases; build the 8 per-core input maps."""
    f32 = np.float32
    q = np.ascontiguousarray(inputs["query"], dtype=f32)
    k = np.ascontiguousarray(inputs["key"], dtype=f32)
    v = np.ascontiguousarray(inputs["value"], dtype=f32)
    w3 = np.asarray(inputs["conv_w3"], f32)
    w5 = np.asarray(inputs["conv_w5"], f32)
    b3 = np.asarray(inputs["conv_b3"], f32)
    b5 = np.asarray(inputs["conv_b5"], f32)
    gam = np.asarray(inputs["bn_gamma"], f32)
    bet = np.asarray(inputs["bn_beta"], f32)
    mu = np.asarray(inputs["bn_mean"], f32)
    var = np.asarray(inputs["bn_var"], f32)
    wq = np.asarray(inputs["wq"], f32)
    bq = np.asarray(inputs["bq"], f32)
    wkl = np.asarray(inputs["wkl"], f32)
    bkl = np.asarray(inputs["bkl"], f32)
    wkg = np.asarray(inputs["wkg"], f32)
    bkg = np.asarray(inputs["bkg"], f32)
    wv = np.asarray(inputs["wv"], f32)
    bv = np.asarray(inputs["bv"], f32)
    wo = np.asarray(inputs["wo"], f32)
    bo = np.asarray(inputs["bo"], f32)

    # biases that would change the math in ways we don't model on-chip
    assert not np.any(bq) and not np.any(bkg), "nonzero q/kg bias unsupported"

    s_bn = gam / np.sqrt(var + BN_EPS)                       # [1024]
    shift = np.concatenate([b3, b5]) * s_bn + (bet - mu * s_bn)
    wkl_s = wkl * s_bn[None, :]                              # [512, 1024]
    A3 = np.einsum("oc,cit->oit", wkl_s[:, :D], w3)          # [512, 512, 3]
    A5 = np.einsum("oc,cit->oit", wkl_s[:, D:], w5)          # [512, 512, 5]
    W5c = A5.copy()
    W5c[:, :, 1:4] += A3
    bkl_eff = wkl @ shift + bkl                              # [512]
    wq_eff = wq / math.sqrt(DK)
    bo_eff = bo + wo @ (2.0 * bv)

    bf = BF16_NP
    in_maps = []
    for c in range(N_CORES):
        b = c // 2
        hg = c % 2
        sel = slice(hg * DO, hg * DO + DO)
        in_maps.append({
            "xq": np.ascontiguousarray(q[b].T).astype(bf),
            "xk": np.ascontiguousarray(k[b].T).astype(bf),
            "xv": np.ascontiguousarray(v[b].T).astype(bf),
            "wq": np.ascontiguousarray(wq_eff.T[:, sel]).astype(bf),
            "wk5": np.ascontiguousarray(W5c.transpose(2, 1, 0)[:, :, sel]).astype(bf),
            "wkg": np.ascontiguousarray(wkg.T[:, sel]).astype(bf),
            "wv": np.ascontiguousarray(wv.T[:, sel]).astype(bf),
            "wo": np.ascontiguousarray(wo.T[sel, :]).astype(bf),
            "bkl": np.ascontiguousarray(bkl_eff[sel]).astype(f32),
        })
    return in_maps, bo_eff


def kernel(**inputs) -> np.ndarray:
    if "nc" not in _cache:
        _cache["nc"] = _build_program()
    nc = _cache["nc"]
    in_maps, bo_eff = _host_prep(inputs)
    res = bass_utils.run_bass_kernel_spmd(
        nc, in_maps, core_ids=list(range(N_CORES)))
    out = np.zeros((B, L, D), np.float32)
    for c in range(N_CORES):
        out[c // 2] += res.results[c]["out"]
    out += bo_eff[None, None, :]
    return out

